# revision 35
# baseline (speedup 1.0000x reference)
"""Trainium2 Bass kernel for a pre-norm transformer block (B=4, L=2048, D=1024,
H=16, hd=64, F=4096, causal attention with additive rel-pos bias).

Sharding: 8 cores, zero collectives. Core c -> batch b = c//2, parity p = c%2.
Each core processes 8 query blocks (128 rows each) of its batch, interleaved
by parity (q-block i = 2j + p for slot j), so causal load is balanced.

v2 design notes (vs v1):
- Scores are computed per (head, key-tile) with a single wide matmul over all
  query slots that need that key tile (contiguous columns [j0*128, 1024),
  j0 = kti//2), split at the 512-column PSUM bank boundary into two q-half
  passes (C0: cols 0:512 / kti 0..7, C1: cols 512:1024 / kti 0..15).
- The rel-pos bias (with causal mask folded in, host-side) is accumulated
  into the scores PSUM by an identity-weights matmul - no vector-engine
  scale/bias op in the softmax inner loop. The 1/sqrt(hd) scale is folded
  into wq/bq host-side.
- exp runs once per (head, key-tile) on the wide tile; P (exp'd scores) lands
  in persistent per-kti SBUF buffers whose causally-dead columns are zeroed
  once at startup, so PV accumulation is uniform full-width with simple
  start/stop flags.
- Z comes from an all-ones column appended to V (row 64 of the PV output);
  normalization is DVE reciprocal + gpsimd partition_broadcast + one DVE
  multiply that also casts attn-out to bf16.
- FFN SiLU is computed as x*0.5*(1+tanh(x/2)) (exact), with the 0.5 folded
  into w2 host-side, so every table-based activation in the kernel (exp,
  tanh, square, copy, identity) lives in the same activation table set and
  only the three tiny rmsnorm Sqrt ops force a table switch.
- All bulk DMAs are batched and issued from the gpsimd queue.
"""

import sys

sys.path.insert(0, "/opt/trn_rl_repo")

import numpy as np
import ml_dtypes

import concourse.bass as bass
import concourse.mybir as mybir
import concourse.tile as tile
from concourse import bacc
from concourse.bass_utils import run_bass_kernel_spmd
from concourse.masks import make_identity

BF = ml_dtypes.bfloat16
FP32 = mybir.dt.float32
BF16 = mybir.dt.bfloat16
AF = mybir.ActivationFunctionType
ALU = mybir.AluOpType

B, L, D, H, HD, F = 4, 2048, 1024, 16, 64, 4096
MASK_VAL = -30000.0

# C0 pass: q cols [0, 512), key tiles 0..7, score width W0 = 512 - j0*128
KTI0 = 8
W0 = [512 - (k // 2) * 128 for k in range(KTI0)]
CUM0 = [sum(W0[:k]) for k in range(KTI0)]
TOT0 = sum(W0)                       # 2560
# C1 pass: q cols [512, 1024), key tiles 0..15
KTI1 = 16
C1S = [max(512, (k // 2) * 128) for k in range(KTI1)]   # global col start
W1 = [1024 - c for c in C1S]
CUM1 = [sum(W1[:k]) for k in range(KTI1)]
TOT1 = sum(W1)                       # 6656
BIAS_PER_HEAD = TOT0 + TOT1          # 9216
BIAS_TOT = H * 128 * BIAS_PER_HEAD


def build_nc():
    nc = bacc.Bacc(None, target_bir_lowering=False)

    xt = nc.dram_tensor("xt", [D, L], BF16, kind="ExternalInput")
    xq = nc.dram_tensor("xq", [D, 1024], BF16, kind="ExternalInput")
    xres = nc.dram_tensor("xres", [D, 1024], FP32, kind="ExternalInput")
    biast = nc.dram_tensor("biast", [BIAS_TOT], BF16, kind="ExternalInput")
    wqt = nc.dram_tensor("wqt", [D, D], BF16, kind="ExternalInput")
    wkt = nc.dram_tensor("wkt", [D, D], BF16, kind="ExternalInput")
    wvt = nc.dram_tensor("wvt", [D, D], BF16, kind="ExternalInput")
    wot = nc.dram_tensor("wot", [D, D], BF16, kind="ExternalInput")
    w1t = nc.dram_tensor("w1t", [D, F], BF16, kind="ExternalInput")
    w2t = nc.dram_tensor("w2t", [F, D], BF16, kind="ExternalInput")
    bq = nc.dram_tensor("bq", [D], FP32, kind="ExternalInput")
    bk = nc.dram_tensor("bk", [D], FP32, kind="ExternalInput")
    bv = nc.dram_tensor("bv", [D], FP32, kind="ExternalInput")
    bo = nc.dram_tensor("bo", [D], FP32, kind="ExternalInput")
    b1h = nc.dram_tensor("b1h", [F], FP32, kind="ExternalInput")  # 0.5*b1
    b1f = nc.dram_tensor("b1f", [F], FP32, kind="ExternalInput")  # b1
    b2 = nc.dram_tensor("b2", [D], FP32, kind="ExternalInput")
    outp = nc.dram_tensor("outp", [1024, D], FP32, kind="ExternalOutput")

    def emit(tc):
        # norm1 output (outlives pxin; allocated first for stack order)
        pht = tc.alloc_tile_pool(name="pht", bufs=1, side="right")
        ht = pht.tile([128, 8, L], BF16)     # rmsnorm(x)^T, full batch
        hq = pht.tile([128, 8, 1024], BF16)  # rmsnorm(x)^T, q-cols only

        # input x streams first (biggest early dependency)
        pxin = tc.alloc_tile_pool(name="pxin", bufs=1, side="right")
        xs = pxin.tile([128, 8, L], BF16)
        xqs = pxin.tile([128, 8, 1024], BF16)
        for c in range(8):
            nc.sync.dma_start(xs[:, c], xt[c * 128:(c + 1) * 128, :])
            nc.sync.dma_start(xqs[:, c], xq[c * 128:(c + 1) * 128, :])

        # ---------------- constants ------------------------------------
        pconst = tc.alloc_tile_pool(name="pconst", bufs=1, side="left")
        ones1 = pconst.tile([128, 1], BF16)
        nc.vector.memset(ones1[:], 1.0)
        identB = pconst.tile([128, 128], BF16)
        make_identity(nc, identB[:])
        identT = pconst.tile([128, 128], FP32)
        make_identity(nc, identT[:])
        bqt = pconst.tile([128, 8], FP32)
        nc.sync.dma_start(bqt[:], bq.rearrange("(g p) -> p g", p=128))
        bkt = pconst.tile([128, 8], FP32)
        nc.sync.dma_start(bkt[:], bk.rearrange("(g p) -> p g", p=128))
        bot = pconst.tile([128, 8], FP32)
        nc.sync.dma_start(bot[:], bo.rearrange("(g p) -> p g", p=128))
        b1ht = pconst.tile([128, 32], FP32)
        nc.sync.dma_start(b1ht[:], b1h.rearrange("(g p) -> p g", p=128))
        b1ft = pconst.tile([128, 32], FP32)
        nc.sync.dma_start(b1ft[:], b1f.rearrange("(g p) -> p g", p=128))
        b2t = pconst.tile([128, 8], FP32)
        nc.sync.dma_start(b2t[:], b2.rearrange("(g p) -> p g", p=128))
        epsb = pconst.tile([1, 1], FP32)
        nc.vector.memset(epsb[:], 1e-6)
        bvrow = pconst.tile([1, 1024], FP32)
        nc.sync.dma_start(bvrow[:], bv[None, :])
        bvB = pconst.tile([128, 1024], FP32)
        nc.gpsimd.partition_broadcast(bvB[:], bvrow[:])

        # ---------------- Phase A: norm1 -------------------------------
        with tc.tile_pool(name="psq", bufs=2) as psq, \
             tc.tile_pool(name="pnrm", bufs=1) as pnrm, \
             tc.tile_pool(name="ppsA", bufs=1, space="PSUM") as ppsA:
            ssq = ppsA.tile([1, L], FP32)
            ssqq = ppsA.tile([1, 1024], FP32)
            for c in range(8):
                sq = psq.tile([128, L], BF16, tag="sq")
                nc.vector.tensor_mul(sq[:], xs[:, c], xs[:, c])
                for s in range(0, L, 512):
                    nc.tensor.matmul(ssq[:, s:s + 512], ones1[:],
                                     sq[:, s:s + 512],
                                     start=(c == 0), stop=(c == 7))
                sqq = psq.tile([128, 1024], BF16, tag="sqq")
                nc.vector.tensor_mul(sqq[:], xqs[:, c], xqs[:, c])
                for s in range(0, 1024, 512):
                    nc.tensor.matmul(ssqq[:, s:s + 512], ones1[:],
                                     sqq[:, s:s + 512],
                                     start=(c == 0), stop=(c == 7))
            rms = pnrm.tile([1, L], FP32)
            nc.scalar.activation(rms[:], ssq[:], AF.Sqrt,
                                 scale=1.0 / D, bias=epsb[:])
            rinv = pnrm.tile([1, L], BF16)
            with nc.allow_low_precision(reason="rstd broadcast in bf16"):
                nc.vector.reciprocal(rinv[:], rms[:])
            rB = pnrm.tile([128, L], BF16)
            nc.gpsimd.partition_broadcast(rB[:], rinv[:])
            rmsq = pnrm.tile([1, 1024], FP32)
            nc.scalar.activation(rmsq[:], ssqq[:], AF.Sqrt,
                                 scale=1.0 / D, bias=epsb[:])
            rinvq = pnrm.tile([1, 1024], BF16)
            with nc.allow_low_precision(reason="rstd broadcast in bf16"):
                nc.vector.reciprocal(rinvq[:], rmsq[:])
            rBq = pnrm.tile([128, 1024], BF16)
            nc.gpsimd.partition_broadcast(rBq[:], rinvq[:])
            for c in range(8):
                nc.vector.tensor_mul(ht[:, c, :], xs[:, c], rB[:])
            for c in range(8):
                nc.vector.tensor_mul(hq[:, c, :], xqs[:, c], rBq[:])

        pxin.release()

        # ---------------- Phase B: Q, K (feature-major), V (row-major) --
        pkv = tc.alloc_tile_pool(name="pkv", bufs=1, side="left")
        kt = pkv.tile([128, 8, L], BF16)           # K^T [feat, key]
        qt = pkv.tile([128, 8, 1024], BF16)        # Q^T [feat, query]
        vv = pkv.tile([128, 16, 16, 65], BF16)     # V rows [key, (h, hd+1)]
        nc.vector.memset(vv[:, :, :, 64:65], 1.0)

        with tc.tile_pool(name="pw2", bufs=3) as pw2, \
             tc.tile_pool(name="pwv", bufs=1) as pwv, \
             tc.tile_pool(name="pps2", bufs=4, space="PSUM") as pps2:
            for g in range(8):
                wkg = pw2.tile([128, 8, 128], BF16, tag="wkg")
                nc.sync.dma_start(
                    wkg[:], wkt[:, g * 128:(g + 1) * 128]
                    .rearrange("(c p) o -> p c o", p=128))
                for s in range(0, L, 512):
                    pk = pps2.tile([128, 512], FP32, tag="pp", name="pk")
                    for c in range(8):
                        nc.tensor.matmul(pk[:], wkg[:, c], ht[:, c, s:s + 512],
                                         start=(c == 0), stop=(c == 7))
                    nc.scalar.activation(kt[:, g, s:s + 512], pk[:],
                                         AF.Identity, bias=bkt[:, g:g + 1])
                wqg = pw2.tile([128, 8, 128], BF16, tag="wqg")
                nc.sync.dma_start(
                    wqg[:], wqt[:, g * 128:(g + 1) * 128]
                    .rearrange("(c p) o -> p c o", p=128))
                for s in range(0, 1024, 512):
                    pq = pps2.tile([128, 512], FP32, tag="pp", name="pq")
                    for c in range(8):
                        nc.tensor.matmul(pq[:], wqg[:, c], hq[:, c, s:s + 512],
                                         start=(c == 0), stop=(c == 7))
                    nc.scalar.activation(qt[:, g, s:s + 512], pq[:],
                                         AF.Identity, bias=bqt[:, g:g + 1])
            wvs = pwv.tile([128, 8, 1024], BF16, tag="wvs")
            for c in range(8):
                nc.sync.dma_start(wvs[:, c], wvt[c * 128:(c + 1) * 128, :])
            for lt in range(16):
                for hf in range(2):
                    pv = pps2.tile([128, 512], FP32, tag="pp", name="pv")
                    for c in range(8):
                        nc.tensor.matmul(
                            pv[:], ht[:, c, lt * 128:(lt + 1) * 128],
                            wvs[:, c, hf * 512:(hf + 1) * 512],
                            start=(c == 0), stop=(c == 7))
                    nc.vector.tensor_add(
                        vv[:, lt, hf * 8:(hf + 1) * 8, 0:64],
                        pv[:].rearrange("p (h e) -> p h e", e=64),
                        bvB[:, hf * 512:(hf + 1) * 512]
                        .rearrange("p (h e) -> p h e", e=64))

        pht.release()

        # ---------------- Phase C: attention ----------------------------
        pao = tc.alloc_tile_pool(name="pao", bufs=1, side="right")
        aoT = pao.tile([128, 8, 1024], BF16)   # attn-out^T [feat, query]

        # residual streams in during attention (consumed in phase D)
        pxr = tc.alloc_tile_pool(name="pxr", bufs=1, side="right")
        xr = pxr.tile([128, 8, 1024], FP32)

        # --- both q-half passes merged into one software-pipelined stream --
        PASS = [
            dict(kti_n=KTI0, wlist=W0, cumlist=CUM0, qbase=0, bias_off=0),
            dict(kti_n=KTI1, wlist=W1, cumlist=CUM1, qbase=512,
                 bias_off=H * 128 * TOT0),
        ]
        for pa in PASS:
            pa["tot"] = pa["cumlist"][-1] + pa["wlist"][-1]
            pa["chunks"] = [(ks, min(ks + 4, pa["kti_n"]))
                            for ks in range(0, pa["kti_n"], 4)]
        pP = tc.alloc_tile_pool(name="pP", bufs=1, side="right")
        Ps = [pP.tile([128, KTI0, 512], BF16, name="P0"),
              pP.tile([128, KTI1, 512], BF16, name="P1")]
        for np_, pa in enumerate(PASS):
            for k in range(pa["kti_n"]):
                dead = 512 - pa["wlist"][k]
                if dead > 0:
                    nc.vector.memset(Ps[np_][:, k, 0:dead], 0.0)

        steps = [(np_, h, k)
                 for np_, pa in enumerate(PASS)
                 for h in range(H) for k in range(pa["kti_n"])]
        gchunks = [(np_, h, ks, ke)
                   for np_, pa in enumerate(PASS)
                   for h in range(H) for ks, ke in pa["chunks"]]
        chunk_of = {}
        for gi, (np_, h, ks, ke) in enumerate(gchunks):
            chunk_of[(np_, h, ks)] = gi

        with tc.tile_pool(name="pbias", bufs=4) as pbias, \
             tc.tile_pool(name="pz", bufs=2) as pz, \
             tc.tile_pool(name="pS", bufs=3, space="PSUM") as pS, \
             tc.tile_pool(name="pPo", bufs=2, space="PSUM") as pPo:

            def finish_head(key, pout):
                np_, fh = key
                fhg, fhp = fh // 2, 64 * (fh % 2)
                qb = PASS[np_]["qbase"]
                zrec = pz.tile([1, 512], FP32, tag="zr", name="zr")
                nc.vector.reciprocal(zrec[:], pout[64:65, :])
                zbB = pz.tile([64, 512], FP32, tag="zb", name="zb")
                nc.gpsimd.partition_broadcast(zbB[:], zrec[:])
                nc.vector.tensor_mul(
                    aoT[fhp:fhp + 64, fhg, qb:qb + 512],
                    pout[0:64, :], zbB[:])

            bsts = {}

            def bias_chunk_dma(gi):
                np_, h, ks, ke = gchunks[gi]
                pa = PASS[np_]
                cw = (pa["cumlist"][ke - 1] + pa["wlist"][ke - 1]
                      - pa["cumlist"][ks])
                bst = pbias.tile([128, 2048], BF16, tag="bst", name="bst")
                off = (pa["bias_off"] + h * 128 * pa["tot"]
                       + 128 * pa["cumlist"][ks])
                nc.sync.dma_start(
                    bst[:, :cw],
                    biast[off:off + 128 * cw]
                    .rearrange("(p w) -> p w", p=128))
                bsts[(np_, h, ks)] = bst

            for gi in range(3):
                bias_chunk_dma(gi)
            next_chunk = [3]
            pouts = {}
            pending = []

            def emit_pv(pnp, ph, pk):
                pa = PASS[pnp]
                pw = pa["wlist"][pk]
                pd = 512 - pw
                nc.tensor.matmul(pouts[(pnp, ph)][:, pd:512],
                                 vv[:, pk, ph], Ps[pnp][:, pk, pd:512],
                                 start=(pk == 0),
                                 stop=(pk == pa["kti_n"] - 1),
                                 skip_group_check=True)
                if pk == pa["kti_n"] - 1:
                    finish_head((pnp, ph), pouts.pop((pnp, ph)))

            S2 = None
            for np_, h, k in steps:
                pa = PASS[np_]
                kti_n, wlist, cumlist = pa["kti_n"], pa["wlist"], pa["cumlist"]
                qbase = pa["qbase"]
                hg, hp = h // 2, 64 * (h % 2)
                if np_ == 1 and h < 8 and k == 0:
                    nc.sync.dma_start(xr[:, h],
                                      xres[h * 128:(h + 1) * 128, :])
                if k == 0:
                    pouts[(np_, h)] = pPo.tile([65, 512], FP32, tag="po",
                                               name="po")
                if k % 4 == 0 and next_chunk[0] < len(gchunks):
                    bias_chunk_dma(next_chunk[0])
                    next_chunk[0] += 1
                w = wlist[k]
                dead = 512 - w
                ks = (k // 4) * 4
                boff = cumlist[k] - cumlist[ks]
                if k % 2 == 0:
                    S2 = pS.tile([128, 2, 512], FP32, tag="S", name="S2")
                half = k % 2
                nc.tensor.matmul(
                    S2[:, half, 0:w],
                    kt[hp:hp + 64, hg, k * 128:(k + 1) * 128],
                    qt[hp:hp + 64, hg, qbase + dead:qbase + 512],
                    start=True, stop=False)
                nc.tensor.matmul(
                    S2[:, half, 0:w], identB[:],
                    bsts[(np_, h, ks)][:, boff:boff + w],
                    start=False, stop=True)
                if half == 1 or k == kti_n - 1:
                    # one exp per key-tile pair (pair widths are equal)
                    nc.scalar.activation(Ps[np_][:, k - half:k + 1, dead:512],
                                         S2[:, 0:half + 1, 0:w], AF.Exp)
                pending.append((np_, h, k))
                if len(pending) > 2:
                    emit_pv(*pending.pop(0))
            for pnp, ph, pk in pending:
                emit_pv(pnp, ph, pk)
        pP.release()

        pkv.release()

        # ---------------- Phase D: W_O + FFN, per q-half ----------------
        pwo = tc.alloc_tile_pool(name="pwo", bufs=1, side="right")
        wov = pwo.tile([128, 8, 8, 128], BF16)   # all of wo resident
        for g in range(8):
            nc.sync.dma_start(
                wov[:, g], wot[:, g * 128:(g + 1) * 128]
                .rearrange("(c p) o -> p c o", p=128))
        px2 = tc.alloc_tile_pool(name="px2", bufs=1, side="left")
        x2T = px2.tile([128, 8, 1024], FP32)

        # stage1 = W_O + residual + norm2 + h2 for one q-half.
        # stage2 = FFN1 + FFN2 + transpose + store for one q-half.
        # stage1(1) is emitted interleaved into stage2's FFN1(0) so the
        # serial norm2 chain of half 1 hides under half-0 FFN compute.
        ph2 = tc.alloc_tile_pool(name="ph2", bufs=1, side="left")
        h2 = ph2.tile([128, 2, 8, 512], BF16)

        def stage1(n, ppsW, ppsS, pnrm2, psq2):
            q0 = n * 512
            sfx = "h%d" % n
            for g in range(8):
                po = ppsW.tile([128, 512], FP32, tag="wo", name="po" + sfx)
                for c in range(8):
                    nc.tensor.matmul(po[:], wov[:, g, c],
                                     aoT[:, c, q0:q0 + 512],
                                     start=(c == 0), stop=(c == 7))
                nc.vector.scalar_tensor_tensor(
                    x2T[:, g, q0:q0 + 512], po[:], bot[:, g:g + 1],
                    xr[:, g, q0:q0 + 512], op0=ALU.add, op1=ALU.add)
            ssq = ppsS.tile([1, 512], FP32, tag="ssq", name="ssq" + sfx)
            for c in range(8):
                sq = psq2.tile([128, 512], BF16, tag="sq", name="sq" + sfx)
                nc.scalar.activation(sq[:], x2T[:, c, q0:q0 + 512], AF.Square)
                nc.tensor.matmul(ssq[:], ones1[:], sq[:],
                                 start=(c == 0), stop=(c == 7))
            rms = pnrm2.tile([1, 512], FP32, tag="rms", name="rms" + sfx)
            nc.scalar.activation(rms[:], ssq[:], AF.Sqrt,
                                 scale=1.0 / D, bias=epsb[:])
            rinv = pnrm2.tile([1, 512], BF16, tag="rinv", name="rinv" + sfx)
            with nc.allow_low_precision(reason="rstd broadcast in bf16"):
                nc.vector.reciprocal(rinv[:], rms[:])
            rB = pnrm2.tile([128, 512], BF16, tag="rB", name="rB" + sfx)
            nc.gpsimd.partition_broadcast(rB[:], rinv[:])
            for c in range(8):
                nc.vector.tensor_mul(h2[:, n, c, :], x2T[:, c, q0:q0 + 512],
                                     rB[:])

        def stage2(n, pwf, pu, pot, pf1, ppsD):
            q0 = n * 512
            sfx = "f%d" % n
            f1T = pf1.tile([128, 32, 512], BF16, tag="f1", bufs=2,
                           name="f1T" + sfx)
            for gf in range(32):
                w1g = pwf.tile([128, 8, 128], BF16, tag="w1g",
                               name="w1g" + sfx)
                nc.sync.dma_start(
                    w1g[:], w1t[:, gf * 128:(gf + 1) * 128]
                    .rearrange("(c p) o -> p c o", p=128))
                z = ppsD.tile([128, 512], FP32, tag="pp", name="z" + sfx)
                for c in range(8):
                    nc.tensor.matmul(z[:], w1g[:, c], h2[:, n, c, :],
                                     start=(c == 0), stop=(c == 7))
                u = pu.tile([128, 512], BF16, tag="u", name="u" + sfx)
                nc.scalar.activation(u[:], z[:], AF.Tanh,
                                     scale=0.5, bias=b1ht[:, gf:gf + 1])
                t2 = pu.tile([128, 512], BF16, tag="t2", name="t2" + sfx)
                nc.vector.scalar_tensor_tensor(
                    t2[:], z[:], b1ft[:, gf:gf + 1], u[:],
                    op0=ALU.add, op1=ALU.mult)
                nc.vector.scalar_tensor_tensor(
                    f1T[:, gf, :], z[:], b1ft[:, gf:gf + 1], t2[:],
                    op0=ALU.add, op1=ALU.add)
            # FFN2, with transposes software-pipelined one g behind
            prev = None
            for g in range(8):
                w2g = pwf.tile([128, 32, 128], BF16, tag="w2g", bufs=3,
                               name="w2g" + sfx)
                nc.sync.dma_start(
                    w2g[:], w2t[:, g * 128:(g + 1) * 128]
                    .rearrange("(c p) o -> p c o", p=128))
                o = ppsD.tile([128, 512], FP32, tag="pp", name="o" + sfx)
                for cf in range(32):
                    nc.tensor.matmul(o[:], w2g[:, cf], f1T[:, cf, :],
                                     start=(cf == 0), stop=(cf == 31))
                og = pot.tile([128, 512], FP32, tag="og", name="og" + sfx)
                nc.vector.scalar_tensor_tensor(
                    og[:], o[:], b2t[:, g:g + 1],
                    x2T[:, g, q0:q0 + 512], op0=ALU.add, op1=ALU.add)
                if prev is not None:
                    emit_store(n, prev[0], prev[1])
                prev = (g, og)
            emit_store(n, prev[0], prev[1])

        def emit_store(n, g, og):
            sfx = "f%d" % n
            with tc.tile_pool(name="ptr%d%d" % (n, g), bufs=2) as ptr, \
                 tc.tile_pool(name="ppst%d%d" % (n, g), bufs=2,
                              space="PSUM") as ppst:
                for j in range(4):
                    pt = ppst.tile([128, 128], FP32, tag="pt",
                                   name="pt" + sfx)
                    nc.tensor.transpose(pt[:], og[:, j * 128:(j + 1) * 128],
                                        identT[:])
                    ot = ptr.tile([128, 128], FP32, tag="ot", name="ot" + sfx)
                    nc.vector.tensor_copy(ot[:], pt[:])
                    nc.gpsimd.dma_start(
                        outp[(n * 4 + j) * 128:(n * 4 + j + 1) * 128,
                             g * 128:(g + 1) * 128],
                        ot[:])

        with tc.tile_pool(name="psq2", bufs=2) as psq2, \
             tc.tile_pool(name="pnrm2", bufs=1) as pnrm2, \
             tc.tile_pool(name="ppsW", bufs=3, space="PSUM") as ppsW, \
             tc.tile_pool(name="ppsS", bufs=1, space="PSUM") as ppsS:
            stage1(0, ppsW, ppsS, pnrm2, psq2)
            stage1(1, ppsW, ppsS, pnrm2, psq2)

        # aoT / xr / wov fully consumed; free them before the FFN pools
        pwo.release()
        pxr.release()
        pao.release()

        with tc.tile_pool(name="pwf", bufs=3) as pwf, \
             tc.tile_pool(name="pu", bufs=3) as pu, \
             tc.tile_pool(name="pot", bufs=2) as pot, \
             tc.tile_pool(name="pf1", bufs=1) as pf1, \
             tc.tile_pool(name="ppsD", bufs=4, space="PSUM") as ppsD:
            stage2(0, pwf, pu, pot, pf1, ppsD)
            stage2(1, pwf, pu, pot, pf1, ppsD)

        ph2.release()
        px2.release()
        pconst.release()

    with tile.TileContext(nc, pool_alloc_mode="queue") as tc:
        emit(tc)

    nc.compile()
    return nc


_NC = None


def _get_nc():
    global _NC
    if _NC is None:
        _NC = build_nc()
    return _NC


def _prep_core_inputs(c, x, rel_pos_bias, wq, bq, wk, bk, wv, bv, wo, bo,
                      norm1_w, norm2_w, w1, b1, w2, b2):
    b, par = c // 2, c % 2
    xb = x[b]                                   # [L, D] f32
    qrows = np.concatenate(
        [np.arange(128 * (2 * j + par), 128 * (2 * j + par) + 128)
         for j in range(8)])
    rel = np.asarray(rel_pos_bias[0], dtype=np.float32)   # [H, L, L]

    bias_flat = np.empty(BIAS_TOT, dtype=BF)
    # layout: per pass, per head, per 4-key-tile chunk: contiguous [128, cw]
    # (matches the kernel's chunked staging DMAs)
    off = 0
    for npass, (kti_n, wlist, cumlist, qbase) in enumerate(
            [(KTI0, W0, CUM0, 0), (KTI1, W1, CUM1, 512)]):
        tot = cumlist[-1] + wlist[-1]
        blk = np.empty((H, 128, tot), dtype=np.float32)
        for k in range(kti_n):
            w = wlist[k]
            dead = 512 - w
            qcols = qbase + dead + np.arange(w)           # local q col idx
            j = qcols // 128                              # slot
            i = 2 * j + par                               # q block
            qglob = i * 128 + (qcols % 128)               # global q row
            k0 = k * 128
            sub = rel[:, qglob, k0:k0 + 128]              # [H, w, 128]
            mask = (k0 + np.arange(128))[None, :] > qglob[:, None]  # [w,128]
            sub = np.where(mask[None], MASK_VAL, sub)
            blk[:, :, cumlist[k]:cumlist[k] + w] = sub.transpose(0, 2, 1)
        parts = []
        for ks in range(0, kti_n, 4):
            ke = min(ks + 4, kti_n)
            cw = cumlist[ke - 1] + wlist[ke - 1] - cumlist[ks]
            parts.append(blk[:, :, cumlist[ks]:cumlist[ks] + cw]
                         .reshape(H, -1))
        bias_flat[off:off + H * 128 * tot] = \
            np.concatenate(parts, axis=1).reshape(-1).astype(BF)
        off += H * 128 * tot

    scale = HD ** -0.5
    return {
        "xt": np.ascontiguousarray(xb.T).astype(BF),
        "xq": np.ascontiguousarray(xb[qrows].T).astype(BF),
        "xres": np.ascontiguousarray(xb[qrows].T).astype(np.float32),
        "biast": bias_flat,
        "wqt": np.ascontiguousarray(wq.T * scale
                                    * norm1_w[:, None]).astype(BF),
        "wkt": np.ascontiguousarray(wk.T * norm1_w[:, None]).astype(BF),
        "wvt": np.ascontiguousarray(wv.T * norm1_w[:, None]).astype(BF),
        "wot": np.ascontiguousarray(wo.T).astype(BF),
        "w1t": np.ascontiguousarray(w1.T * norm2_w[:, None]).astype(BF),
        "w2t": np.ascontiguousarray(w2.T * 0.5).astype(BF),
        "bq": (bq * scale).astype(np.float32),
        "bk": bk.astype(np.float32),
        "bv": bv.astype(np.float32), "bo": bo.astype(np.float32),
        "b1h": (0.5 * b1).astype(np.float32),
        "b1f": b1.astype(np.float32),
        "b2": b2.astype(np.float32),
    }


def kernel(**inputs):
    inputs = {k: np.asarray(v) for k, v in inputs.items()}
    nc = _get_nc()
    in_maps = [_prep_core_inputs(c, **inputs) for c in range(8)]
    res = run_bass_kernel_spmd(nc, in_maps, core_ids=list(range(8)))
    out = np.empty((B, L, D), dtype=np.float32)
    for c in range(8):
        b, par = c // 2, c % 2
        o = res.results[c]["outp"]
        for j in range(8):
            i = 2 * j + par
            out[b, 128 * i:128 * i + 128] = o[128 * j:128 * j + 128]
    return out
